# revision 1
# baseline (speedup 1.0000x reference)
"""Causal single-head attention (B=4, T=2048, E=1024, H=128) on 8 NeuronCores.

Sharding: core = (batch b, half h). Each core computes the attention output for
one half (1024 rows) of one batch. Per-core input xt is x[b]^T with the key
column-blocks permuted so the core's OWN query half always sits at columns
1024:2048 (so the SPMD program is identical across cores); causality for the
permuted key order is enforced via per-core additive mask DATA, not code.

Pipeline per core (all on-chip, one launch):
  xT (f32r) --PE--> qT,kT,vT [H,T] (f32r)   (W stationary, x moving, E chunked)
  vT --PE transpose--> v natural [T,H] (fp16, + ones column for denominators)
  scores^T [keys,q] = kT_blk^T @ qT_grp (f32r, N=256)  + additive causal masks
  expS = exp(scores^T) (ACT, fp16)
  out[q,H] | denom[q,1] = sum_kb expS_blk^T @ [v|1]  (fp16 matmul, fp32 PSUM)
  y = out * (1/denom)  (DVE)
Softmax skips max-subtraction: scores ~ N(0,1), exp is safe in fp32/fp16 range.
"""

import math

import numpy as np

import concourse.bass as bass
import concourse.tile as tile
from concourse import bacc, mybir
from concourse.bass_utils import run_bass_kernel_spmd
from concourse.masks import make_identity

B, T, E, H = 4, 2048, 1024, 128
NB = T // 128          # 16 key slots
NE = E // 128          # 8 contraction chunks
NG = 4                 # q groups of 256 (= 8 local q blocks)
BIG = 1.0e30

F32 = mybir.dt.float32
F32R = mybir.dt.float32r
F16 = mybir.dt.float16

_CACHE: dict = {}


def _build():
    nc = bacc.Bacc(None, target_bir_lowering=False)
    xt = nc.dram_tensor("xt", [E, T], F32R, kind="ExternalInput")
    wq = nc.dram_tensor("wq", [E, H], F32R, kind="ExternalInput")
    wk = nc.dram_tensor("wk", [E, H], F32R, kind="ExternalInput")
    wv = nc.dram_tensor("wv", [E, H], F32R, kind="ExternalInput")
    bq = nc.dram_tensor("bq", [H, 1], F32, kind="ExternalInput")
    bk = nc.dram_tensor("bk", [H, 1], F32, kind="ExternalInput")
    msk = nc.dram_tensor("msk", [128, 2304], F32, kind="ExternalInput")
    y = nc.dram_tensor("y", [T // 2, H], F32, kind="ExternalOutput")

    with tile.TileContext(nc) as tc:
        with (
            tc.tile_pool(name="xs", bufs=1) as xs_pool,
            tc.tile_pool(name="wsb", bufs=1) as w_pool,
            tc.tile_pool(name="small", bufs=1) as sm_pool,
            tc.tile_pool(name="qk", bufs=1) as qk_pool,
            tc.tile_pool(name="ex", bufs=8) as ex_pool,
            tc.tile_pool(name="ob", bufs=3) as ob_pool,
        ):
            xs = xs_pool.tile([128, NE * T], F32R)
            for e in range(NE):
                for qr in range(4):
                    eng = nc.gpsimd if (e * 4 + qr) % 2 == 0 else nc.sync
                    eng.dma_start(
                        xs[:, e * T + qr * (T // 4): e * T + (qr + 1) * (T // 4)],
                        xt[e * 128:(e + 1) * 128, qr * (T // 4):(qr + 1) * (T // 4)])
            wsb = w_pool.tile([128, 3 * E], F32R)
            for p, w in enumerate((wq, wk, wv)):
                for e in range(NE):
                    nc.sync.dma_start(
                        wsb[:, p * E + e * 128: p * E + (e + 1) * 128],
                        w[e * 128:(e + 1) * 128, :])
            bq_sb = sm_pool.tile([128, 1], F32, tag="bq")
            bk_sb = sm_pool.tile([128, 1], F32, tag="bk")
            nc.sync.dma_start(bq_sb[:], bq[:])
            nc.sync.dma_start(bk_sb[:], bk[:])
            mask_sb = sm_pool.tile([128, 2304], F32, tag="msk")
            nc.sync.dma_start(mask_sb[:], msk[:])
            ident = sm_pool.tile([128, 128], F32, tag="ident")
            make_identity(nc, ident[:])

            qT = qk_pool.tile([128, T // 2], F32R, tag="qT")
            kTs = [qk_pool.tile([128, 512], F32R, name=f"kT{t}", tag=f"kT{t}") for t in range(4)]
            vTs = [qk_pool.tile([128, 512], F32, name=f"vT{t}", tag=f"vT{t}") for t in range(4)]
            vaugs = [qk_pool.tile([128, 4 * 129], F16, name=f"vaug{t}", tag=f"vaug{t}") for t in range(4)]
            for t in range(4):
                nc.vector.memset(vaugs[t][:], 1.0)

            def kT(kb):
                return kTs[kb // 4][:, (kb % 4) * 128:(kb % 4 + 1) * 128]

            def vaug(kb):
                return vaugs[kb // 4][:, (kb % 4) * 129:(kb % 4 + 1) * 129]

            # ---- projections + v transposes, interleaved per 512-slot so
            # attention groups unblock as early as possible
            with (
                tc.tile_pool(name="pps", bufs=3, space="PSUM") as pps,
                tc.tile_pool(name="tps", bufs=2, space="PSUM") as tps,
            ):
                # qT first: every attention group needs it
                for t in range(2):
                    ps = pps.tile([128, 512], F32)
                    for e in range(NE):
                        nc.tensor.matmul(
                            ps[:],
                            wsb[:, e * 128:(e + 1) * 128],
                            xs[:, e * T + 1024 + t * 512: e * T + 1024 + (t + 1) * 512],
                            start=(e == 0), stop=(e == NE - 1))
                    nc.scalar.activation(
                        qT[:, t * 512:(t + 1) * 512], ps[:],
                        mybir.ActivationFunctionType.Identity, bias=bq_sb[:])
                for t in range(4):
                    ps = pps.tile([128, 512], F32)
                    for e in range(NE):
                        nc.tensor.matmul(
                            ps[:],
                            wsb[:, E + e * 128: E + (e + 1) * 128],
                            xs[:, e * T + t * 512: e * T + (t + 1) * 512],
                            start=(e == 0), stop=(e == NE - 1))
                    nc.scalar.activation(
                        kTs[t][:], ps[:],
                        mybir.ActivationFunctionType.Identity, bias=bk_sb[:])
                    ps = pps.tile([128, 512], F32)
                    for e in range(NE):
                        nc.tensor.matmul(
                            ps[:],
                            wsb[:, 2 * E + e * 128: 2 * E + (e + 1) * 128],
                            xs[:, e * T + t * 512: e * T + (t + 1) * 512],
                            start=(e == 0), stop=(e == NE - 1))
                    nc.vector.tensor_copy(vTs[t][:], ps[:])
                    for s in range(4):
                        tp = tps.tile([128, 128], F32)
                        nc.tensor.transpose(tp[:], vTs[t][:, s * 128:(s + 1) * 128], ident[:])
                        nc.vector.tensor_copy(
                            vaugs[t][:, s * 129: s * 129 + 128], tp[:])

            # ---- attention: group j covers q cols [256j, 256j+256)
            with (
                tc.tile_pool(name="sps", bufs=3, space="PSUM") as sps,
                tc.tile_pool(name="avps", bufs=2, space="PSUM") as avps,
            ):
                for j in range(2):
                    kj = 12 + 4 * j
                    av = [avps.tile([128, 129], F32, name=f"av{q}", tag=f"av{q}", bufs=1) for q in range(4)]
                    for kb in range(kj):
                        sp = sps.tile([128, 512], F32)
                        nc.tensor.matmul(
                            sp[:], kT(kb),
                            qT[:, j * 512:(j + 1) * 512], start=True, stop=True)
                        ebias = 0.0
                        if kb < 8:
                            # M_low is per-partition constant: fold into exp bias
                            ebias = mask_sb[:, 0:1]
                        elif kb >= kj - 4:
                            r = kb - (kj - 4)
                            nc.vector.tensor_add(
                                sp[:], sp[:], mask_sb[:, 256 + r * 512: 256 + (r + 1) * 512])
                        ex = ex_pool.tile([128, 512], F16)
                        nc.scalar.activation(ex[:], sp[:], mybir.ActivationFunctionType.Exp,
                                             bias=ebias)
                        for q in range(4):
                            nc.tensor.matmul(
                                av[q][:],
                                ex[:, q * 128:(q + 1) * 128],
                                vaug(kb),
                                start=(kb == 0), stop=(kb == kj - 1))
                    for q in range(4):
                        rc = ob_pool.tile([128, 1], F32, tag="rc")
                        nc.vector.reciprocal(rc[:], av[q][:, 128:129])
                        ob = ob_pool.tile([128, 128], F32, tag="ob")
                        nc.vector.tensor_scalar_mul(ob[:], av[q][:, 0:128], rc[:])
                        nc.scalar.dma_start(y[(4 * j + q) * 128:(4 * j + q + 1) * 128, :], ob[:])
    nc.compile()
    return nc


def _masks(h: int) -> np.ndarray:
    p = np.arange(128)[:, None]  # key within block (partition)
    c = np.arange(128)[None, :]  # query within block (free)
    tri = np.where(p <= c, 0.0, -BIG).astype(np.float32)
    z = np.zeros((128, 128), np.float32)
    inv = np.full((128, 128), -BIG, np.float32)
    m_low = (z if h == 1 else inv)[:, :128]
    parts = [np.concatenate([m_low, m_low], axis=1)]  # cols 0:256 (only col 0 used)
    for r in range(4):  # window mask W_r for the last 4 kbs of a 512-q group
        quarters = [z if r < cq else (tri if r == cq else inv) for cq in range(4)]
        parts.append(np.concatenate(quarters, axis=1))
    return np.ascontiguousarray(np.concatenate(parts, axis=1))


def kernel(x, Wq, bq, Wk, bk, Wv, bv):
    x = np.asarray(x, dtype=np.float32)
    Wq = np.asarray(Wq, dtype=np.float32)
    Wk = np.asarray(Wk, dtype=np.float32)
    Wv = np.asarray(Wv, dtype=np.float32)
    bq = np.asarray(bq, dtype=np.float32)
    bk = np.asarray(bk, dtype=np.float32)
    bv = np.asarray(bv, dtype=np.float32)

    if "nc" not in _CACHE:
        _CACHE["nc"] = _build()
    nc = _CACHE["nc"]

    scale = 1.0 / math.sqrt(H)
    wq_s = np.ascontiguousarray(Wq * scale)
    bq_s = np.ascontiguousarray((bq * scale).reshape(H, 1))
    bk_r = np.ascontiguousarray(bk.reshape(H, 1))
    masks = {0: _masks(0), 1: _masks(1)}

    xt = np.ascontiguousarray(x.transpose(0, 2, 1))  # [B, E, T]
    in_maps = []
    for core in range(8):
        b, h = divmod(core, 2)
        if h == 1:
            xtc = xt[b]
        else:
            xtc = np.ascontiguousarray(
                np.concatenate([xt[b][:, T // 2:], xt[b][:, :T // 2]], axis=1))
        in_maps.append({
            "xt": xtc, "wq": wq_s, "wk": Wk, "wv": Wv,
            "bq": bq_s, "bk": bk_r, "msk": masks[h],
        })

    res = run_bass_kernel_spmd(nc, in_maps, core_ids=list(range(8)))
    out = np.empty((B, T, H), dtype=np.float32)
    for core in range(8):
        b, h = divmod(core, 2)
        out[b, h * (T // 2):(h + 1) * (T // 2), :] = res.results[core]["y"]
    out += bv  # sum_j softmax_ij = 1, so +bv commutes with attention
    return out



# revision 4
# speedup vs baseline: 2.0442x; 2.0442x over previous
"""Causal single-head attention (B=4, T=2048, E=1024, H=128) on 8 NeuronCores.

Sharding: core = (batch b, parity h). Block-cyclic over query blocks: core h of
batch b owns q-blocks {h, h+2, ..., h+14} (8 blocks of 128 rows), so causal
score work is near-balanced (64 vs 72 key-block units) instead of the 1:3 skew
of a contiguous split. Stage s handles local q-block s (global 2s+h) and needs
key slots 0..2s+1.

Per-core SBUF x layout puts each pair of key blocks {2s+h, 2s+1-h} with the
core's OWN block first, so the SPMD program is identical across cores; the
causal mask for the last two key slots is per-core DATA ([tri|-BIG] on even
cores, [tri|0] on odd).

Everything is fp16 on the PE (1 cycle/row at any tile size, halves HBM bytes);
accumulation stays fp32 in PSUM. Per stage:
  kT[:, 2s*128:+256]  = Wk^T x  (PE, W stationary, x moving) + bias  (ACT, fp16)
  qT[:, s*128:+128]   = Wq^T x own block (+ bias, pre-scaled 1/sqrt(H))
  v natural           = x^T Wv (PE, x stationary -> no transposes), copied to
                        vaug [pos, H | 1] (DVE, fp16)
  scores^T chunks     = kT_slot^T @ qT_s  (PE, [keys,128q] per slot, 4/bank)
  exp                 (ACT, fp16; no max-subtraction: scores ~ N(0,1))
  out|denom           = sum_kb expS_kb^T @ vaug_kb  (PE, fp32 PSUM)
  y = out * (1/denom) (DVE), DMA out per q-block.

DMA order: small consts on the gpsimd queue; Wk, xpair0, Wq, Wv, xpair1.. on
the sync queue so the first k-projection starts ~3us in and compute overlaps
the rest of the load. bv is added on the host after gather (softmax rows sum
to 1, so +bv commutes).
"""

import math

import numpy as np

import concourse.bass as bass
import concourse.tile as tile
from concourse import bacc, mybir
from concourse.bass_utils import run_bass_kernel_spmd

B, T, E, H = 4, 2048, 1024, 128
NB = T // 128        # 16 key slots
NE = E // 128        # 8 contraction chunks
NS = 8               # stages (local q-blocks) per core
BIG = 1.0e30

F32 = mybir.dt.float32
F16 = mybir.dt.float16

_CACHE: dict = {}


def _build():
    nc = bacc.Bacc(None, target_bir_lowering=False)
    xd = nc.dram_tensor("xd", [128, NE * T], F16, kind="ExternalInput")
    wd = nc.dram_tensor("wd", [128, 3 * E], F16, kind="ExternalInput")
    bqd = nc.dram_tensor("bqd", [128, 1], F32, kind="ExternalInput")
    bkd = nc.dram_tensor("bkd", [128, 1], F32, kind="ExternalInput")
    md = nc.dram_tensor("md", [128, 256], F32, kind="ExternalInput")
    y = nc.dram_tensor("y", [NS * 128, H], F32, kind="ExternalOutput")

    WK, WQ, WV = 0, E, 2 * E  # wsb column offsets

    with tile.TileContext(nc) as tc:
        with (
            tc.tile_pool(name="xs", bufs=1) as xs_pool,
            tc.tile_pool(name="ws", bufs=1) as w_pool,
            tc.tile_pool(name="small", bufs=1) as sm_pool,
            tc.tile_pool(name="qk", bufs=1) as qk_pool,
            tc.tile_pool(name="ex", bufs=4) as ex_pool,
            tc.tile_pool(name="ob", bufs=2) as ob_pool,
            tc.tile_pool(name="pp", bufs=2, space="PSUM") as pp_pool,
            tc.tile_pool(name="pv1", bufs=2, space="PSUM") as pv1_pool,
            tc.tile_pool(name="sps", bufs=2, space="PSUM") as sps_pool,
            tc.tile_pool(name="avps", bufs=2, space="PSUM") as av_pool,
        ):
            xs = xs_pool.tile([128, NE * T], F16)
            wsb = w_pool.tile([128, 3 * E], F16)
            bq_sb = sm_pool.tile([128, 1], F32, tag="bq")
            bk_sb = sm_pool.tile([128, 1], F32, tag="bk")
            mask_sb = sm_pool.tile([128, 256], F32, tag="msk")
            kT = qk_pool.tile([128, T], F16, tag="kT")
            qT = qk_pool.tile([128, NS * 128], F16, tag="qT")
            vaug = qk_pool.tile([128, NB * 129], F16, tag="vaug")

            # small consts on the gpsimd (SWDGE) queue; bulk on sync (SP)
            nc.gpsimd.dma_start(bk_sb[:], bkd[:])
            nc.gpsimd.dma_start(bq_sb[:], bqd[:])
            nc.gpsimd.dma_start(mask_sb[:], md[:])
            nc.vector.memset(vaug[:], 1.0)  # ones column for denominators
            nc.sync.dma_start(wsb[:, WK:WK + E], wd[:, WK:WK + E])
            nc.sync.dma_start(xs[:, 0:2048], xd[:, 0:2048])
            nc.sync.dma_start(wsb[:, WQ:WQ + E], wd[:, WQ:WQ + E])
            nc.sync.dma_start(wsb[:, WV:WV + E], wd[:, WV:WV + E])
            for s in range(1, NS):
                nc.sync.dma_start(
                    xs[:, s * 2048:(s + 1) * 2048], xd[:, s * 2048:(s + 1) * 2048])

            def emit_proj(s):
                x0 = s * 2048
                # one PSUM bank per stage: k scores [0:256], q [256:384],
                # v block 0 [384:512]; v block 1 in its own half-bank pool
                pp = pp_pool.tile([128, 512], F32)
                pk, pq, pv0 = pp[:, 0:256], pp[:, 256:384], pp[:, 384:512]
                pv1 = pv1_pool.tile([128, 128], F32)
                for e in range(NE):
                    nc.tensor.matmul(
                        pk, wsb[:, WK + e * 128:WK + (e + 1) * 128],
                        xs[:, x0 + e * 256:x0 + (e + 1) * 256],
                        start=(e == 0), stop=(e == NE - 1))
                nc.scalar.activation(
                    kT[:, 2 * s * 128:(2 * s + 2) * 128], pk,
                    mybir.ActivationFunctionType.Identity, bias=bk_sb[:])
                for e in range(NE):
                    nc.tensor.matmul(
                        pq, wsb[:, WQ + e * 128:WQ + (e + 1) * 128],
                        xs[:, x0 + e * 256:x0 + e * 256 + 128],
                        start=(e == 0), stop=(e == NE - 1))
                nc.scalar.activation(
                    qT[:, s * 128:(s + 1) * 128], pq,
                    mybir.ActivationFunctionType.Identity, bias=bq_sb[:])
                for blk, pv in ((0, pv0), (1, pv1[:, 0:128])):
                    for e in range(NE):
                        nc.tensor.matmul(
                            pv,
                            xs[:, x0 + e * 256 + blk * 128:x0 + e * 256 + (blk + 1) * 128],
                            wsb[:, WV + e * 128:WV + (e + 1) * 128],
                            start=(e == 0), stop=(e == NE - 1))
                    kb = 2 * s + blk
                    nc.vector.tensor_copy(
                        vaug[:, kb * 129:kb * 129 + 128], pv)

            def emit_attn(s):
                n = 2 * s + 2                       # key slots for this stage
                chunks = [(c0, min(c0 + 4, n)) for c0 in range(0, n, 4)]
                av = av_pool.tile([128, 129], F32)
                sps, exs = [], []

                def emit_score(ci):
                    c0, c1 = chunks[ci]
                    sp = sps_pool.tile([128, 512], F32)
                    sps.append(sp)
                    for kb in range(c0, c1):
                        m = kb - c0
                        nc.tensor.matmul(
                            sp[:, m * 128:(m + 1) * 128],
                            kT[:, kb * 128:(kb + 1) * 128],
                            qT[:, s * 128:(s + 1) * 128],
                            start=True, stop=True)
                    w = (c1 - c0) * 128
                    if c1 == n:  # causal masks live on the last two slots
                        nc.vector.tensor_add(
                            sp[:, w - 256:w], sp[:, w - 256:w], mask_sb[:])
                    ex = ex_pool.tile([128, 512], F16)
                    exs.append(ex)
                    nc.scalar.activation(
                        ex[:, 0:w], sp[:, 0:w], mybir.ActivationFunctionType.Exp)

                def emit_av(ci):
                    c0, c1 = chunks[ci]
                    for kb in range(c0, c1):
                        m = kb - c0
                        nc.tensor.matmul(
                            av[:], exs[ci][:, m * 128:(m + 1) * 128],
                            vaug[:, kb * 129:(kb + 1) * 129],
                            start=(kb == 0), stop=(kb == n - 1))

                emit_score(0)
                for ci in range(1, len(chunks)):
                    emit_score(ci)
                    emit_av(ci - 1)
                emit_av(len(chunks) - 1)

                rc = ob_pool.tile([128, 1], F32, tag="rc")
                nc.vector.reciprocal(rc[:], av[:, 128:129])
                ob = ob_pool.tile([128, 128], F32, tag="ob")
                nc.vector.tensor_scalar_mul(ob[:], av[:, 0:128], rc[:])
                nc.sync.dma_start(y[s * 128:(s + 1) * 128, :], ob[:])

            for s in range(NS):
                emit_proj(s)
                emit_attn(s)
    nc.compile()
    return nc


def _pack_w(w: np.ndarray) -> np.ndarray:
    # [E, H] -> [128, E]: chunk e at cols e*128, partitions = rows e*128+p
    return w.reshape(NE, 128, H).transpose(1, 0, 2).reshape(128, E)


def _mask(h: int) -> np.ndarray:
    p = np.arange(128)[:, None]  # key position within slot (partition)
    c = np.arange(128)[None, :]  # query position within block (free)
    tri = np.where(p <= c, 0.0, -BIG).astype(np.float32)
    last = np.full((128, 128), -BIG, np.float32) if h == 0 else np.zeros((128, 128), np.float32)
    return np.ascontiguousarray(np.concatenate([tri, last], axis=1))


def kernel(x, Wq, bq, Wk, bk, Wv, bv):
    x = np.asarray(x, dtype=np.float32)
    Wq = np.asarray(Wq, dtype=np.float32)
    Wk = np.asarray(Wk, dtype=np.float32)
    Wv = np.asarray(Wv, dtype=np.float32)
    bq = np.asarray(bq, dtype=np.float32)
    bk = np.asarray(bk, dtype=np.float32)
    bv = np.asarray(bv, dtype=np.float32)

    if "nc" not in _CACHE:
        _CACHE["nc"] = _build()
    nc = _CACHE["nc"]

    scale = 1.0 / math.sqrt(H)
    wsb = np.ascontiguousarray(np.concatenate(
        [_pack_w(Wk), _pack_w(Wq * scale), _pack_w(Wv)], axis=1)).astype(np.float16)
    bq_s = np.ascontiguousarray((bq * scale).reshape(H, 1))
    bk_r = np.ascontiguousarray(bk.reshape(H, 1))
    masks = {0: _mask(0), 1: _mask(1)}

    # [b, blk, pos, e, ep]
    xb = x.astype(np.float16).reshape(B, NB, 128, NE, 128)
    in_maps = []
    for core in range(8):
        b, h = divmod(core, 2)
        own = xb[b, h::2]       # [8, pos, e, ep] blocks h, h+2, ...
        oth = xb[b, 1 - h::2]
        pair = np.stack([own, oth], axis=1)          # [s, which, pos, e, ep]
        xsc = np.ascontiguousarray(
            pair.transpose(4, 0, 3, 1, 2).reshape(128, NE * T))
        in_maps.append({
            "xd": xsc, "wd": wsb, "bqd": bq_s, "bkd": bk_r, "md": masks[h],
        })

    res = run_bass_kernel_spmd(nc, in_maps, core_ids=list(range(8)))
    out = np.empty((B, T, H), dtype=np.float32)
    for core in range(8):
        b, h = divmod(core, 2)
        yc = res.results[core]["y"]
        for s in range(NS):
            g = 2 * s + h
            out[b, g * 128:(g + 1) * 128, :] = yc[s * 128:(s + 1) * 128, :]
    out += bv  # softmax rows sum to 1, so +bv commutes with attention
    return out


# revision 7
# speedup vs baseline: 2.1718x; 1.0624x over previous
"""Causal single-head attention (B=4, T=2048, E=1024, H=128) on 8 NeuronCores.

Sharding: core = (batch b, parity h). Block-cyclic over query blocks: core h of
batch b owns q-blocks {h, h+2, ..., h+14} (8 blocks of 128 rows), so causal
score work is near-balanced (64 vs 72 key-block units) instead of the 1:3 skew
of a contiguous split. Stage s handles local q-block s (global 2s+h) and needs
key slots 0..2s+1.

Per-core SBUF x layout puts each pair of key blocks {2s+h, 2s+1-h} with the
core's OWN block first, so the SPMD program is identical across cores; the
causal mask for the last two key slots is per-core DATA ([tri|-BIG] on even
cores, [tri|0] on odd).

Everything is fp16 on the PE (1 cycle/row at any tile size, halves HBM bytes);
accumulation stays fp32 in PSUM. Per stage:
  kT[:, 2s*128:+256]  = Wk^T x  (PE, W stationary, x moving) + bias  (DVE, fp16)
  qT[:, s*128:+128]   = Wq^T x own block (+ bias, pre-scaled 1/sqrt(H); ACT)
  v natural           = x^T Wv (PE, x stationary -> no transposes), copied to
                        vaug [pos, H | 1] (DVE, fp16)
  scores^T chunks     = kT_slot^T @ qT_s  (PE, [keys,128q] per slot, 4/bank);
                        the masked chunk (last two slots) is computed FIRST so
                        the mask-add/exp never sit on the stage's drain path
  exp                 (ACT, fp16; no max-subtraction: scores ~ N(0,1))
  out|denom           = sum_kb expS_kb^T @ vaug_kb  (PE, fp32 PSUM)
  y = out * (1/denom) (DVE), DMA out per q-block.

Emission runs projections one stage ahead of attention so PE always has
projection work to cover exp latency. DMA: consts on the gpsimd queue, the
first x pair on the scalar queue (parallel DGE spin-up), weights + remaining
pairs on sync, ordered so the first k-projection starts ~2.5us in. bv is
added on the host after gather (softmax rows sum to 1, so +bv commutes).
"""

import math

import numpy as np

import concourse.bass as bass
import concourse.tile as tile
from concourse import bacc, mybir
from concourse.bass_utils import run_bass_kernel_spmd

B, T, E, H = 4, 2048, 1024, 128
NB = T // 128        # 16 key slots
NE = E // 128        # 8 contraction chunks
NS = 8               # stages (local q-blocks) per core
BIG = 1.0e30

F32 = mybir.dt.float32
F16 = mybir.dt.float16

_CACHE: dict = {}


def _build():
    nc = bacc.Bacc(None, target_bir_lowering=False)
    xd = nc.dram_tensor("xd", [128, NE * T], F16, kind="ExternalInput")
    wd = nc.dram_tensor("wd", [128, 3 * E], F16, kind="ExternalInput")
    bqd = nc.dram_tensor("bqd", [128, 1], F32, kind="ExternalInput")
    bkd = nc.dram_tensor("bkd", [128, 1], F32, kind="ExternalInput")
    md = nc.dram_tensor("md", [128, 256], F32, kind="ExternalInput")
    y = nc.dram_tensor("y", [NS * 128, H], F32, kind="ExternalOutput")

    WK, WQ, WV = 0, E, 2 * E  # wsb column offsets

    with tile.TileContext(nc) as tc:
        with (
            tc.tile_pool(name="xs", bufs=1) as xs_pool,
            tc.tile_pool(name="ws", bufs=1) as w_pool,
            tc.tile_pool(name="small", bufs=1) as sm_pool,
            tc.tile_pool(name="qk", bufs=1) as qk_pool,
            tc.tile_pool(name="ex", bufs=4) as ex_pool,
            tc.tile_pool(name="ob", bufs=2) as ob_pool,
            tc.tile_pool(name="pp", bufs=2, space="PSUM") as pp_pool,
            tc.tile_pool(name="sps", bufs=3, space="PSUM") as sps_pool,
            tc.tile_pool(name="pv1", bufs=2, space="PSUM") as pv1_pool,
            tc.tile_pool(name="avp", bufs=1, space="PSUM") as av_pool,
        ):
            xs = xs_pool.tile([128, NE * T], F16)
            wsb = w_pool.tile([128, 3 * E], F16)
            bq_sb = sm_pool.tile([128, 1], F32, tag="bq")
            bk_sb = sm_pool.tile([128, 1], F32, tag="bk")
            mask_sb = sm_pool.tile([128, 256], F32, tag="msk")
            kT = qk_pool.tile([128, T], F16, tag="kT")
            qT = qk_pool.tile([128, NS * 128], F16, tag="qT")
            vaug = qk_pool.tile([128, NB * 129], F16, tag="vaug")

            # small consts on the gpsimd (SWDGE) queue; first x pair on the
            # scalar queue so its DGE pipeline spins up in parallel with
            # sync's; weights + remaining pairs on sync.
            nc.gpsimd.dma_start(bk_sb[:], bkd[:])
            nc.gpsimd.dma_start(bq_sb[:], bqd[:])
            nc.gpsimd.dma_start(mask_sb[:], md[:])
            nc.vector.memset(vaug[:], 1.0)  # ones column for denominators
            nc.scalar.dma_start(xs[:, 0:1024], xd[:, 0:1024])
            nc.scalar.dma_start(xs[:, 1024:2048], xd[:, 1024:2048])
            nc.sync.dma_start(wsb[:, WK:WK + E], wd[:, WK:WK + E])
            nc.sync.dma_start(wsb[:, WQ:WQ + E], wd[:, WQ:WQ + E])
            nc.sync.dma_start(wsb[:, WV:WV + E], wd[:, WV:WV + E])
            for s in range(1, NS):
                nc.sync.dma_start(
                    xs[:, s * 2048:(s + 1) * 2048], xd[:, s * 2048:(s + 1) * 2048])

            def emit_proj(s):
                x0 = s * 2048
                # one PSUM bank per stage: k scores [0:256], q [256:384],
                # v block 0 [384:512]; v block 1 in its own half-bank slot
                pp = pp_pool.tile([128, 512], F32)
                pk, pq, pv0 = pp[:, 0:256], pp[:, 256:384], pp[:, 384:512]
                pv1 = pv1_pool.tile([128, 128], F32)
                for e in range(NE):
                    nc.tensor.matmul(
                        pk, wsb[:, WK + e * 128:WK + (e + 1) * 128],
                        xs[:, x0 + e * 256:x0 + (e + 1) * 256],
                        start=(e == 0), stop=(e == NE - 1))
                nc.vector.tensor_scalar_add(
                    kT[:, 2 * s * 128:(2 * s + 2) * 128], pk, bk_sb[:])
                for e in range(NE):
                    nc.tensor.matmul(
                        pq, wsb[:, WQ + e * 128:WQ + (e + 1) * 128],
                        xs[:, x0 + e * 256:x0 + e * 256 + 128],
                        start=(e == 0), stop=(e == NE - 1))
                nc.scalar.activation(
                    qT[:, s * 128:(s + 1) * 128], pq,
                    mybir.ActivationFunctionType.Identity, bias=bq_sb[:])
                for blk, pv in ((0, pv0), (1, pv1[:])):
                    for e in range(NE):
                        nc.tensor.matmul(
                            pv,
                            xs[:, x0 + e * 256 + blk * 128:x0 + e * 256 + (blk + 1) * 128],
                            wsb[:, WV + e * 128:WV + (e + 1) * 128],
                            start=(e == 0), stop=(e == NE - 1))
                    kb = 2 * s + blk
                    nc.vector.tensor_copy(
                        vaug[:, kb * 129:kb * 129 + 128], pv)

            def emit_attn(s):
                n = 2 * s + 2                       # key slots for this stage
                chunks = [(c0, min(c0 + 4, n)) for c0 in range(0, n, 4)]
                C = len(chunks)
                # masked chunk (holding slots 2s, 2s+1) first; stop-matmul
                # lands on an unmasked chunk so exp/mask never gate the drain
                order = [C - 1] + list(range(C - 1))
                av = av_pool.tile([128, 129], F32)
                exs = {}

                def emit_score(ci):
                    c0, c1 = chunks[ci]
                    w = (c1 - c0) * 128
                    sp = sps_pool.tile([128, 512], F32)
                    for kb in range(c0, c1):
                        m = kb - c0
                        nc.tensor.matmul(
                            sp[:, m * 128:(m + 1) * 128],
                            kT[:, kb * 128:(kb + 1) * 128],
                            qT[:, s * 128:(s + 1) * 128],
                            start=True, stop=True)
                    if c1 == n:  # causal masks live on the last two slots
                        nc.vector.tensor_add(
                            sp[:, w - 256:w], sp[:, w - 256:w], mask_sb[:])
                    ex = ex_pool.tile([128, 512], F16)
                    exs[ci] = ex
                    nc.scalar.activation(
                        ex[:, 0:w], sp[:, 0:w], mybir.ActivationFunctionType.Exp)

                def emit_av(ci, first, last):
                    c0, c1 = chunks[ci]
                    for kb in range(c0, c1):
                        m = kb - c0
                        nc.tensor.matmul(
                            av[:], exs[ci][:, m * 128:(m + 1) * 128],
                            vaug[:, kb * 129:(kb + 1) * 129],
                            start=(first and kb == c0),
                            stop=(last and kb == c1 - 1))

                emit_score(order[0])
                for i in range(1, C):
                    emit_score(order[i])
                    emit_av(order[i - 1], first=(i == 1), last=False)
                emit_av(order[C - 1], first=(C == 1), last=True)

                rc = ob_pool.tile([128, 1], F32, tag="rc")
                nc.vector.reciprocal(rc[:], av[:, 128:129])
                ob = ob_pool.tile([128, 128], F32, tag="ob")
                nc.vector.tensor_scalar_mul(ob[:], av[:, 0:128], rc[:])
                eng = nc.sync if s == NS - 1 else nc.gpsimd
                eng.dma_start(y[s * 128:(s + 1) * 128, :], ob[:])

            emit_proj(0)
            for s in range(NS):
                if s + 1 < NS:
                    emit_proj(s + 1)
                emit_attn(s)
    nc.compile()
    return nc


def _pack_w(w: np.ndarray) -> np.ndarray:
    # [E, H] -> [128, E]: chunk e at cols e*128, partitions = rows e*128+p
    return w.reshape(NE, 128, H).transpose(1, 0, 2).reshape(128, E)


def _mask(h: int) -> np.ndarray:
    p = np.arange(128)[:, None]  # key position within slot (partition)
    c = np.arange(128)[None, :]  # query position within block (free)
    tri = np.where(p <= c, 0.0, -BIG).astype(np.float32)
    last = np.full((128, 128), -BIG, np.float32) if h == 0 else np.zeros((128, 128), np.float32)
    return np.ascontiguousarray(np.concatenate([tri, last], axis=1))


def kernel(x, Wq, bq, Wk, bk, Wv, bv):
    x = np.asarray(x, dtype=np.float32)
    Wq = np.asarray(Wq, dtype=np.float32)
    Wk = np.asarray(Wk, dtype=np.float32)
    Wv = np.asarray(Wv, dtype=np.float32)
    bq = np.asarray(bq, dtype=np.float32)
    bk = np.asarray(bk, dtype=np.float32)
    bv = np.asarray(bv, dtype=np.float32)

    if "nc" not in _CACHE:
        _CACHE["nc"] = _build()
    nc = _CACHE["nc"]

    scale = 1.0 / math.sqrt(H)
    wsb = np.ascontiguousarray(np.concatenate(
        [_pack_w(Wk), _pack_w(Wq * scale), _pack_w(Wv)], axis=1)).astype(np.float16)
    bq_s = np.ascontiguousarray((bq * scale).reshape(H, 1))
    bk_r = np.ascontiguousarray(bk.reshape(H, 1))
    masks = {0: _mask(0), 1: _mask(1)}

    # [b, blk, pos, e, ep]
    xb = x.astype(np.float16).reshape(B, NB, 128, NE, 128)
    in_maps = []
    for core in range(8):
        b, h = divmod(core, 2)
        own = xb[b, h::2]       # [8, pos, e, ep] blocks h, h+2, ...
        oth = xb[b, 1 - h::2]
        pair = np.stack([own, oth], axis=1)          # [s, which, pos, e, ep]
        xsc = np.ascontiguousarray(
            pair.transpose(4, 0, 3, 1, 2).reshape(128, NE * T))
        in_maps.append({
            "xd": xsc, "wd": wsb, "bqd": bq_s, "bkd": bk_r, "md": masks[h],
        })

    res = run_bass_kernel_spmd(nc, in_maps, core_ids=list(range(8)))
    out = np.empty((B, T, H), dtype=np.float32)
    for core in range(8):
        b, h = divmod(core, 2)
        yc = res.results[core]["y"]
        for s in range(NS):
            g = 2 * s + h
            out[b, g * 128:(g + 1) * 128, :] = yc[s * 128:(s + 1) * 128, :]
    out += bv  # softmax rows sum to 1, so +bv commutes with attention
    return out
